# revision 17
# baseline (speedup 1.0000x reference)
"""Trainium2 Bass kernel for nn_CROM_Layer_81140522156285 (moe_routing).

Math restructure (exactly equivalent to the reference, far less work):
  last = x[:, -1, :]
  q    = last @ Wq.T
  qk   = (q @ Wk) / sqrt(D)              # tiny [B, D]
  scores[b, s] = x[b, s, :] . qk[b, :]   # one pass over x  (big, memory-bound)
  attn = softmax(scores)                 # = exp(s) / sum(exp(s)), s is O(1)
  ctx  = (attn[b] @ x[b]) @ Wv.T         # second contraction folded into same pass
  out  = ctx @ expert_W[eid].T + expert_b[eid]
  y    = x with last row replaced by LayerNorm(last + out)

The only work proportional to x (128 MiB) is scores + the attn-weighted sum
of x rows.  Both are fused into a single pass on device: each [128, D] tile
contributes scores via a fused DVE multiply+reduce, exp via ScalarE, and the
unnormalized weighted row-sum + partition-sum-of-weights via TensorE matmuls
accumulated in PSUM.  Sharding: sequence dim S=8192 split 1024-per-core
across 8 cores (softmax partials combine linearly).  Each core returns
[B, D] unnormalized context + [B, 1] partition function; the host combines
(tiny) and applies the remaining [B, D]-sized projections / LayerNorm.
"""

import numpy as np

import concourse.bass as bass
import concourse.tile as tile
from concourse.bass import _add_dep_helper
from concourse import bacc, mybir
from concourse.bass_utils import run_bass_kernel_spmd

B = 4
S = 8192
D = 1024
N_CORES = 8
S_CORE = S // N_CORES      # positions per batch handled by one core
P = 128                    # SBUF partitions
NT = S_CORE // P           # s-tiles of 128 positions per batch per core
CHUNK_NT = 2               # s-tiles per DMA (1 MiB per dma_start)
NCHUNK = NT // CHUNK_NT

_NC = None


def _build_nc():
    nc = bacc.Bacc("TRN2", target_bir_lowering=False, debug=False,
                   num_devices=N_CORES)
    f32 = mybir.dt.float32
    f32r = mybir.dt.float32r
    x_ap = nc.dram_tensor("x", [B, S_CORE, D], f32r, kind="ExternalInput").ap()
    ones_ap = nc.dram_tensor("ones", [P, 2], f32r, kind="ExternalInput").ap()
    qkb_ap = nc.dram_tensor("qkb", [1, B * D], f32, kind="ExternalInput").ap()
    ctx_ap = nc.dram_tensor("ctx_out", [B, D], f32, kind="ExternalOutput").ap()
    z_ap = nc.dram_tensor("z_out", [B, 2], f32, kind="ExternalOutput").ap()

    with tile.TileContext(nc) as tc:
        with (
            tc.tile_pool(name="const", bufs=1) as const_pool,
            tc.tile_pool(name="x", bufs=6) as xpool,
            tc.tile_pool(name="scr", bufs=4) as scrpool,
            tc.tile_pool(name="sc", bufs=8) as scpool,
            tc.tile_pool(name="psum", bufs=2, space="PSUM") as psumpool,
            tc.tile_pool(name="stg", bufs=2) as stgpool,
        ):
            ones = const_pool.tile([P, 2], f32r, tag="ones")
            nc.scalar.dma_start(ones[:], ones_ap[:])
            # one 16 KiB load of qk, then on-chip partition broadcasts on
            # the otherwise-idle GpSimd engine (keeps 2 MiB of replicated
            # data out of the HBM DMA wave)
            qksm = const_pool.tile([1, B * D], f32, tag="qksm")
            nc.sync.dma_start(qksm[:], qkb_ap[:])
            qkts = []
            for qb in range(B):
                t = const_pool.tile([P, D], f32, tag=f"qk{qb}")
                qkts.append(t)
                nc.gpsimd.partition_broadcast(
                    t[:], qksm[0:1, qb * D:(qb + 1) * D])

            # deferred per-batch epilogue: PSUM->SBUF staging + output DMAs,
            # emitted after the NEXT batch's compute so the in-order engine
            # streams don't stall on the stop-matmul at batch transitions
            pending = []

            def flush_pending():
                for ps_c0_, ps_c1_, ps_z_, b_ in pending:
                    stg = stgpool.tile([1, D], f32, tag="stg")
                    stz = stgpool.tile([1, 2], f32, tag="stz")
                    nc.vector.tensor_copy(stg[:, 0:512], ps_c0_[:])
                    nc.vector.tensor_copy(stg[:, 512:1024], ps_c1_[:])
                    nc.vector.tensor_copy(stz[:], ps_z_[:])
                    nc.scalar.dma_start(ctx_ap[b_:b_ + 1, :], stg[:])
                    nc.scalar.dma_start(z_ap[b_:b_ + 1, :], stz[:])
                pending.clear()

            x_dmas = []
            for b in range(B):
                # [P, NT, D] view: (p, n, d) -> x[b, p*NT + n, d]; per
                # partition a chunk of CHUNK_NT rows is HBM-contiguous
                xb = x_ap[b, :, :].rearrange("(p n) d -> p n d", p=P)
                ps_c0 = psumpool.tile([1, 512], f32, tag="c0")
                ps_c1 = psumpool.tile([1, 512], f32, tag="c1")
                ps_z = psumpool.tile([1, 2], f32, tag="z")
                widths = [1, 1] + [CHUNK_NT] * ((NT - 2) // CHUNK_NT) \
                    if b == 0 else [CHUNK_NT] * NCHUNK
                off = 0
                for ci, w in enumerate(widths):
                    xt = xpool.tile([P, CHUNK_NT, D], f32r, tag="xt")
                    dma_i = nc.sync.dma_start(
                        xt[:, 0:w, :], xb[:, off:off + w, :])
                    x_dmas.append(dma_i.ins)
                    for j in range(w):
                        n = off + j
                        prod = scrpool.tile([P, D], f32, tag="prod")
                        nc.vector.tensor_tensor(
                            out=prod[:], in0=xt[:, j, :], in1=qkts[b][:],
                            op=mybir.AluOpType.mult)
                        sc = scpool.tile([P, 1], f32, tag="sc")
                        if (b * NT + n) % 16 == 15:
                            nc.vector.tensor_reduce(
                                sc[:], prod[:], axis=mybir.AxisListType.X,
                                op=mybir.AluOpType.add)
                        else:
                            dump = scrpool.tile([P, D], f32, tag="dump")
                            nc.scalar.activation(
                                dump[:], prod[:],
                                mybir.ActivationFunctionType.Copy,
                                accum_out=sc[:])
                        esc = scpool.tile([P, 1], f32r, tag="esc")
                        nc.scalar.activation(
                            esc[:], sc[:], mybir.ActivationFunctionType.Exp)
                        st, sp = (n == 0), (n == NT - 1)
                        nc.tensor.matmul(ps_c0[:], esc[:], xt[:, j, 0:512],
                                         start=st, stop=sp)
                        nc.tensor.matmul(ps_c1[:], esc[:], xt[:, j, 512:1024],
                                         start=st, stop=sp)
                        nc.tensor.matmul(ps_z[:], esc[:], ones[:],
                                         start=st, stop=sp)
                        if ci == 0 and j == w - 1:
                            flush_pending()
                    off += w
                pending.append((ps_c0, ps_c1, ps_z, b))
            flush_pending()

    nc.compile()
    return nc


def _get_nc():
    global _NC
    if _NC is None:
        _NC = _build_nc()
    return _NC


def kernel(x_emb, Wq, Wk, Wv, expert_W, expert_b, ln_gamma, ln_beta,
           expert_id, _spmd_kwargs=None):
    x = np.ascontiguousarray(np.asarray(x_emb, dtype=np.float32))
    Wq = np.asarray(Wq, dtype=np.float32)
    Wk = np.asarray(Wk, dtype=np.float32)
    Wv = np.asarray(Wv, dtype=np.float32)
    expert_b = np.asarray(expert_b, dtype=np.float32)
    ln_gamma = np.asarray(ln_gamma, dtype=np.float32)
    ln_beta = np.asarray(ln_beta, dtype=np.float32)
    eid = int(np.asarray(expert_id))

    last = x[:, -1, :]                                   # [B, D]
    q = last @ Wq.T                                      # [B, D]
    qk = (q @ Wk) * np.float32(1.0 / np.sqrt(D))         # [B, D]
    qkb = np.ascontiguousarray(qk.reshape(1, B * D), dtype=np.float32)

    in_maps = [
        {"x": np.ascontiguousarray(x[:, c * S_CORE:(c + 1) * S_CORE, :]),
         "qkb": qkb, "ones": np.ones((P, 2), dtype=np.float32)}
        for c in range(N_CORES)
    ]
    res = run_bass_kernel_spmd(_get_nc(), in_maps, core_ids=list(range(N_CORES)),
                               **(_spmd_kwargs or {}))
    ctx_un = np.zeros((B, D), dtype=np.float32)
    z = np.zeros((B, 1), dtype=np.float32)
    for c in range(N_CORES):
        ctx_un += res.results[c]["ctx_out"]
        z += res.results[c]["z_out"][:, 0:1]

    ctx = ctx_un / z                                     # [B, D] attn @ x
    context = ctx @ Wv.T                                 # [B, D]
    We = np.asarray(expert_W[eid], dtype=np.float32)     # [D, D]
    out = context @ We.T + expert_b[eid]                 # [B, D]
    resid = last + out
    mu = resid.mean(axis=-1, keepdims=True, dtype=np.float32)
    diff = resid - mu
    var = np.mean(diff * diff, axis=-1, keepdims=True, dtype=np.float32)
    new_focus = diff / np.sqrt(var + np.float32(1e-5)) * ln_gamma + ln_beta

    y = x.copy()
    y[:, -1, :] = new_focus
    return y


if __name__ == "__main__":
    rng = np.random.default_rng(0)
    xs = {
        "x_emb": rng.standard_normal((B, S, D), dtype=np.float32),
        "Wq": rng.standard_normal((D, D), dtype=np.float32) * 0.02,
        "Wk": rng.standard_normal((D, D), dtype=np.float32) * 0.02,
        "Wv": rng.standard_normal((D, D), dtype=np.float32) * 0.02,
        "expert_W": rng.standard_normal((128, D, D), dtype=np.float32) * 0.02,
        "expert_b": rng.standard_normal((128, D), dtype=np.float32) * 0.02,
        "ln_gamma": np.ones(D, dtype=np.float32),
        "ln_beta": np.zeros(D, dtype=np.float32),
        "expert_id": 7,
    }
    y = kernel(**xs)
    print(y.shape, y.dtype)
